# revision 3
# baseline (speedup 1.0000x reference)
"""Trainium2 Bass kernel for the gnn_message_passing "Context" problem.

Reference computation (N=100000 nodes, E=600000 edges, D=128, B=32 graphs):
    c_V    = scatter_mean(dst_na, batch_id)            # [B, D]
    gate_V = sigmoid(MLP3_V(c_V))                      # [B, D]
    out_n  = dst_na * gate_V[batch_id]                 # [N, D]
    c_V2   = scatter_mean(out_n, batch_id)             # == c_V * gate_V (gate const per segment)
    gate_E = sigmoid(MLP3_E(c_V2))                     # [B, D]
    out_e  = ea * gate_E[batch_id[edge_idx[0]]]        # [E, D]

Strategy: block-shard nodes/edges across 8 cores (overlap trick for
non-divisible sizes).  Each core computes a partial transposed segment sum
[D, B] with one-hot matmuls on the PE, AllReduce (tiny, [128,32]) combines
them, then the two 3-layer MLPs run on-device in transposed layout.  Gating
gathers gate rows via one-hot matmuls (PE) and multiplies on the vector
engine; the one-hot construction is spread across PE (broadcast outer
product), ACT ((seg-s)^2 via per-partition bias) and GpSimd (< 1 compare).
"""

import numpy as np

import concourse.bacc as bacc
import concourse.mybir as mybir
import concourse.tile as tile
from concourse.bass_utils import run_bass_kernel_spmd

F32 = mybir.dt.float32
AF = mybir.ActivationFunctionType
ALU = mybir.AluOpType

N_NODES = 100000
N_EDGES = 600000
D = 128
S = 32          # segments (graphs)
NC = 8          # cores
P = 128

# per-core block sizes (multiples of 128)
BN = 12544      # nodes per core  (98 tiles of 128)
BE = 75008      # edges per core  (586 tiles of 128)
NJ_N = BN // P  # 98
NJ_E = BE // P  # 586
MEGA = 16       # j-tiles per mega tile (2048 rows)


def _megas(nj):
    m = [MEGA] * (nj // MEGA)
    if nj % MEGA:
        m.append(nj % MEGA)
    return m


def _build():
    nc = bacc.Bacc(
        "TRN2",
        target_bir_lowering=False,
        debug=False,
        enable_asserts=False,
        num_devices=NC,
    )

    def inp(name, shape):
        return nc.dram_tensor(name, shape, F32, kind="ExternalInput")

    x_d = inp("x", [BN, D])
    bid_d = inp("bid", [BN])
    ea_d = inp("ea", [BE, D])
    seg_d = inp("seg", [BE])
    inv_d = inp("inv_rep", [P, S])
    w_d = {f"{g}w{i}": inp(f"{g}w{i}", [D, D]) for g in "ve" for i in (1, 2, 3)}
    b_d = {f"{g}b{i}": inp(f"{g}b{i}", [D, 1]) for g in "ve" for i in (1, 2, 3)}
    ident_d = inp("ident", [P, P])
    iota_row_d = inp("iota_row", [P, S])
    niota_d = inp("niota", [P, 1])
    ones_d = inp("ones_row", [1, S])

    ox_d = nc.dram_tensor("out_x", [BN, D], F32, kind="ExternalOutput")
    oe_d = nc.dram_tensor("out_e", [BE, D], F32, kind="ExternalOutput")

    x3 = x_d.ap().rearrange("(j p) d -> p j d", p=P)
    ea3 = ea_d.ap().rearrange("(j p) d -> p j d", p=P)
    ox3 = ox_d.ap().rearrange("(j p) d -> p j d", p=P)
    oe3 = oe_d.ap().rearrange("(j p) d -> p j d", p=P)

    with tile.TileContext(nc) as tc:
        with (
            tc.tile_pool(name="const", bufs=1) as cst,
            tc.tile_pool(name="dram", bufs=1, space="DRAM") as dpool,
        ):
            # ---- constants ----
            def cload(ap, shape, tag):
                t = cst.tile(shape, F32, tag=tag)
                nc.sync.dma_start(t[:], ap)
                return t

            inv_rep = cload(inv_d.ap(), [P, S], "inv_rep")
            ws = {k: cload(v.ap(), [D, D], k) for k, v in w_d.items()}
            bs = {k: cload(v.ap(), [D, 1], k) for k, v in b_d.items()}
            ident = cload(ident_d.ap(), [P, P], "ident")
            iota_row = cload(iota_row_d.ap(), [P, S], "iota_row")
            niota = cload(niota_d.ap(), [P, 1], "niota")
            ones_row = cload(ones_d.ap(), [1, S], "ones_row")

            # resident node block + batch ids
            xres = cst.tile([P, NJ_N * D], F32, tag="xres")
            xres3 = xres[:].rearrange("p (j d) -> p j d", d=D)
            for c in range(7):  # 98 = 7 * 14
                sl = slice(c * 14, (c + 1) * 14)
                nc.sync.dma_start(xres3[:, sl, :], x3[:, sl, :])
            bidcols = cst.tile([P, NJ_N], F32, tag="bidcols")
            nc.sync.dma_start(bidcols[:], bid_d.ap().rearrange("(j p) -> p j", p=P))

            gate_v = cst.tile([S, D], F32, tag="gate_v")
            gate_e = cst.tile([S, D], F32, tag="gate_e")

            # ---- phase 1: partial segsum (transposed) + AllReduce + MLPs ----
            with (
                tc.tile_pool(name="p1", bufs=4) as p1,
                tc.tile_pool(name="p1ps", bufs=1, space="PSUM") as pp1,
                tc.tile_pool(name="mlpps", bufs=2, space="PSUM") as ppm,
            ):
                seg_ps = pp1.tile([P, S], F32, tag="seg_ps")
                for j in range(NJ_N):
                    oh = p1.tile([P, S], F32, tag="oh")
                    nc.vector.tensor_tensor(
                        oh[:],
                        bidcols[:, j : j + 1].to_broadcast([P, S]),
                        iota_row[:],
                        ALU.is_equal,
                    )
                    nc.tensor.matmul(
                        seg_ps[:],
                        xres3[:, j, :],
                        oh[:],
                        start=(j == 0),
                        stop=(j == NJ_N - 1),
                    )
                segT = p1.tile([P, S], F32, tag="segT")
                nc.vector.tensor_copy(segT[:], seg_ps[:])
                ccin = dpool.tile([P, S], F32, tag="ccin")
                ccout = dpool.tile([P, S], F32, tag="ccout")
                nc.gpsimd.dma_start(ccin[:], segT[:])
                nc.gpsimd.collective_compute(
                    "AllReduce",
                    ALU.add,
                    replica_groups=[list(range(NC))],
                    ins=[ccin.opt()],
                    outs=[ccout.opt()],
                )
                segS = p1.tile([P, S], F32, tag="segS")
                nc.gpsimd.dma_start(segS[:], ccout[:])
                cT = p1.tile([P, S], F32, tag="cT")
                nc.vector.tensor_tensor(cT[:], segS[:], inv_rep[:], ALU.mult)

                def mlp(h, g):
                    for i in (1, 2, 3):
                        ps = ppm.tile([P, S], F32, tag="mlp_ps")
                        nc.tensor.matmul(ps[:], ws[f"{g}w{i}"][:], h[:], start=True, stop=True)
                        hn = p1.tile([P, S], F32, tag=f"h_{g}{i}")
                        nc.scalar.activation(
                            hn[:], ps[:],
                            AF.Sigmoid if i == 3 else AF.Relu,
                            bias=bs[f"{g}b{i}"][:],
                        )
                        h = hn
                    return h

                gate_vT = mlp(cT, "v")
                c2T = p1.tile([P, S], F32, tag="c2T")
                nc.vector.tensor_tensor(c2T[:], cT[:], gate_vT[:], ALU.mult)
                gate_eT = mlp(c2T, "e")

                for gT, g in ((gate_vT, gate_v), (gate_eT, gate_e)):
                    pt = ppm.tile([S, P], F32, tag="tp_ps")
                    nc.tensor.transpose(pt[:], gT[:], ident[:])
                    nc.vector.tensor_copy(g[:], pt[:])

            # ---- phase 2/3: gating (nodes reuse resident x, edges stream) ----
            with (
                tc.tile_pool(name="io", bufs=3) as iop,
                tc.tile_pool(name="mid", bufs=2) as midp,
                tc.tile_pool(name="pb", bufs=2, space="PSUM") as pbp,
                tc.tile_pool(name="pg", bufs=2, space="PSUM") as pgp,
            ):
                def gating(megas, gate, srow_sbuf, srow_dram, src3, out3, resident):
                    j0 = 0
                    for J in megas:
                        C = J * D
                        if resident is None:
                            xt = iop.tile([P, MEGA * D], F32, tag="in")
                            xt3 = xt[:].rearrange("p (j d) -> p j d", d=D)
                            nc.sync.dma_start(xt3[:, 0:J, :], src3[:, j0 : j0 + J, :])
                            src = xt[:, 0:C]
                        else:
                            src = resident[:, j0 * D : j0 * D + C]
                        if srow_sbuf is not None:
                            srow = srow_sbuf[:, j0 * D : j0 * D + C]
                        else:
                            st = midp.tile([1, MEGA * D], F32, tag="srow")
                            nc.sync.dma_start(
                                st[0:1, 0:C], srow_dram.ap()[None, j0 * D : j0 * D + C]
                            )
                            srow = st[0:1, 0:C]
                        sq = midp.tile([S, MEGA * D], F32, tag="sq")
                        oht = midp.tile([S, MEGA * D], F32, tag="oht")
                        outt = iop.tile([P, MEGA * D], F32, tag="out")
                        for h0 in range(0, J, 8):
                            hj = min(8, J - h0)
                            hc = hj * D
                            hsl = slice(h0 * D, h0 * D + hc)
                            pb = pbp.tile([S, 8 * D], F32, tag="pb")
                            for q0 in range(0, hc, 512):
                                qc = min(512, hc - q0)
                                nc.tensor.matmul(
                                    pb[:, q0 : q0 + qc],
                                    ones_row[:],
                                    srow[0:1, h0 * D + q0 : h0 * D + q0 + qc],
                                    start=True,
                                    stop=True,
                                )
                            nc.scalar.activation(
                                sq[:, hsl], pb[:, 0:hc], AF.Square, bias=niota[0:S, :]
                            )
                            nc.gpsimd.tensor_scalar(
                                oht[:, hsl], sq[:, hsl], 1.0, None, ALU.is_lt
                            )
                            pg = pgp.tile([P, 8 * D], F32, tag="pg")
                            for jj in range(hj):
                                nc.tensor.matmul(
                                    pg[:, jj * D : (jj + 1) * D],
                                    oht[:, (h0 + jj) * D : (h0 + jj + 1) * D],
                                    gate[:],
                                    start=True,
                                    stop=True,
                                )
                            nc.vector.tensor_tensor(
                                outt[:, hsl], src[:, hsl], pg[:, 0:hc], ALU.mult
                            )
                        ot3 = outt[:].rearrange("p (j d) -> p j d", d=D)
                        nc.scalar.dma_start(out3[:, j0 : j0 + J, :], ot3[:, 0:J, :])
                        j0 += J

                gating(_megas(NJ_N), gate_v, None, bid_d, None, ox3, xres)
                gating(_megas(NJ_E), gate_e, None, seg_d, ea3, oe3, None)

    nc.compile()
    return nc


_NC_CACHE = None
LAST_RESULT = None


def _get_nc():
    global _NC_CACHE
    if _NC_CACHE is None:
        _NC_CACHE = _build()
    return _NC_CACHE


def _shard_starts(total, block):
    starts, cover = [], 0
    for k in range(NC):
        s = min(k * block, total - block)
        starts.append(s)
        cover = max(cover, s + block)
    assert cover == total
    return starts


def kernel(dst_na, ea, edge_idx, batch_id,
           Vg_W1, Vg_b1, Vg_W2, Vg_b2, Vg_W3, Vg_b3,
           Eg_W1, Eg_b1, Eg_W2, Eg_b2, Eg_W3, Eg_b3):
    dst_na = np.ascontiguousarray(np.asarray(dst_na, dtype=np.float32))
    ea = np.ascontiguousarray(np.asarray(ea, dtype=np.float32))
    bid = np.asarray(batch_id).astype(np.int64)
    eidx0 = np.asarray(edge_idx)[0].astype(np.int64)

    seg_e = bid[eidx0].astype(np.float32)
    bid_f = bid.astype(np.float32)

    cnt = np.bincount(bid, minlength=S).astype(np.float32)
    inv = 1.0 / np.maximum(cnt, 1.0)
    inv_rep = np.tile(inv[None, :], (P, 1)).astype(np.float32)

    consts = {
        "inv_rep": inv_rep,
        "ident": np.eye(P, dtype=np.float32),
        "iota_row": np.tile(np.arange(S, dtype=np.float32)[None, :], (P, 1)),
        "niota": -np.arange(P, dtype=np.float32)[:, None].copy(),
        "ones_row": np.ones((1, S), dtype=np.float32),
        "vw1": np.ascontiguousarray(Vg_W1, np.float32),
        "vw2": np.ascontiguousarray(Vg_W2, np.float32),
        "vw3": np.ascontiguousarray(Vg_W3, np.float32),
        "ew1": np.ascontiguousarray(Eg_W1, np.float32),
        "ew2": np.ascontiguousarray(Eg_W2, np.float32),
        "ew3": np.ascontiguousarray(Eg_W3, np.float32),
        "vb1": np.asarray(Vg_b1, np.float32).reshape(D, 1).copy(),
        "vb2": np.asarray(Vg_b2, np.float32).reshape(D, 1).copy(),
        "vb3": np.asarray(Vg_b3, np.float32).reshape(D, 1).copy(),
        "eb1": np.asarray(Eg_b1, np.float32).reshape(D, 1).copy(),
        "eb2": np.asarray(Eg_b2, np.float32).reshape(D, 1).copy(),
        "eb3": np.asarray(Eg_b3, np.float32).reshape(D, 1).copy(),
    }

    nstarts = _shard_starts(N_NODES, BN)
    estarts = _shard_starts(N_EDGES, BE)

    in_maps = []
    cover = 0
    novs = []
    for k in range(NC):
        ns, es = nstarts[k], estarts[k]
        bidk = bid_f[ns : ns + BN].copy()
        ov = max(0, cover - ns)  # rows already covered by earlier cores
        novs.append(ov)
        if ov:
            bidk[:ov] = -1.0  # exclude from segment sum; output discarded
        cover = max(cover, ns + BN)
        in_maps.append(
            {
                "x": dst_na[ns : ns + BN],
                "bid": bidk,
                "ea": ea[es : es + BE],
                "seg": seg_e[es : es + BE],
                **consts,
            }
        )

    nc = _get_nc()
    res = run_bass_kernel_spmd(nc, in_maps, core_ids=list(range(NC)))
    global LAST_RESULT
    LAST_RESULT = res

    out_x = np.empty((N_NODES, D), dtype=np.float32)
    out_e = np.empty((N_EDGES, D), dtype=np.float32)
    for k in range(NC):
        ns, es = nstarts[k], estarts[k]
        ov = novs[k]
        out_x[ns + ov : ns + BN] = res.results[k]["out_x"][ov:]
        out_e[es : es + BE] = res.results[k]["out_e"]
    return (out_x, out_e)


# revision 10
# speedup vs baseline: 3.5878x; 3.5878x over previous
"""Trainium2 Bass kernel for the gnn_message_passing "Context" problem.

Reference computation (N=100000 nodes, E=600000 edges, D=128, B=32 graphs):
    c_V    = scatter_mean(dst_na, batch_id)            # [B, D]
    gate_V = sigmoid(MLP3_V(c_V))                      # [B, D]
    out_n  = dst_na * gate_V[batch_id]                 # [N, D]
    c_V2   = scatter_mean(out_n, batch_id)             # == c_V * gate_V (gate const per segment)
    gate_E = sigmoid(MLP3_E(c_V2))                     # [B, D]
    out_e  = ea * gate_E[batch_id[edge_idx[0]]]        # [E, D]

Strategy: block-shard nodes/edges across 8 cores (overlap trick for
non-divisible sizes).  Each core computes a partial transposed segment sum
[D, B] with one-hot matmuls on the PE, AllReduce (tiny, [128,32]) combines
them, then the two 3-layer MLPs run on-device in transposed layout.  Gating
gathers gate rows via one-hot matmuls (PE) and multiplies on the vector
engine; the one-hot construction is spread across PE (broadcast outer
product), ACT ((seg-s)^2 via per-partition bias) and GpSimd (< 1 compare).
"""

import ml_dtypes
import numpy as np

import concourse.bacc as bacc
import concourse.mybir as mybir
import concourse.tile as tile
from concourse.bass_utils import run_bass_kernel_spmd

F32 = mybir.dt.float32
BF16 = mybir.dt.bfloat16
AF = mybir.ActivationFunctionType
ALU = mybir.AluOpType

N_NODES = 100000
N_EDGES = 600000
D = 128
S = 32          # segments (graphs)
NC = 8          # cores
P = 128

# per-core block sizes (multiples of 128)
BN = 12544      # nodes per core  (98 tiles of 128)
BE = 75008      # edges per core  (586 tiles of 128)
NJ_N = BN // P  # 98
NJ_E = BE // P  # 586
MEGA = 16       # j-tiles per mega tile (2048 rows)


def _megas(nj):
    m = [MEGA] * (nj // MEGA)
    if nj % MEGA:
        m.append(nj % MEGA)
    return m


def _build():
    nc = bacc.Bacc(
        "TRN2",
        target_bir_lowering=False,
        debug=False,
        enable_asserts=False,
        num_devices=NC,
    )

    def inp(name, shape):
        return nc.dram_tensor(name, shape, F32, kind="ExternalInput")

    x_d = inp("x", [BN, D])
    bid_d = inp("bid", [BN])
    bidb_d = nc.dram_tensor("bid_bf", [BN], BF16, kind="ExternalInput")
    ea_d = inp("ea", [BE, D])
    segb_d = nc.dram_tensor("seg_bf", [BE], BF16, kind="ExternalInput")
    inv_d = inp("inv_rep", [P, S])
    w_d = {f"{g}w{i}": inp(f"{g}w{i}", [D, D]) for g in "ve" for i in (1, 2, 3)}
    b_d = {f"{g}b{i}": inp(f"{g}b{i}", [D, 1]) for g in "ve" for i in (1, 2, 3)}
    ident_d = inp("ident", [P, P])
    iota_row_d = inp("iota_row", [P, S])
    iota_col_d = inp("iota_col", [P, 1])

    ox_d = nc.dram_tensor("out_x", [BN, D], F32, kind="ExternalOutput")
    oe_d = nc.dram_tensor("out_e", [BE, D], F32, kind="ExternalOutput")

    x3 = x_d.ap().rearrange("(j p) d -> p j d", p=P)
    ea3 = ea_d.ap().rearrange("(j p) d -> p j d", p=P)
    ox3 = ox_d.ap().rearrange("(j p) d -> p j d", p=P)
    oe3 = oe_d.ap().rearrange("(j p) d -> p j d", p=P)

    with tile.TileContext(nc) as tc:
        with (
            tc.tile_pool(name="const", bufs=1) as cst,
            tc.tile_pool(name="dram", bufs=1, space="DRAM") as dpool,
        ):
            # ---- constants ----
            def cload(ap, shape, tag):
                t = cst.tile(shape, F32, tag=tag)
                nc.sync.dma_start(t[:], ap)
                return t

            inv_rep = cload(inv_d.ap(), [P, S], "inv_rep")
            ws = {k: cload(v.ap(), [D, D], k) for k, v in w_d.items()}
            bs = {k: cload(v.ap(), [D, 1], k) for k, v in b_d.items()}
            ident = cload(ident_d.ap(), [P, P], "ident")
            iota_row = cload(iota_row_d.ap(), [P, S], "iota_row")
            iota_col = cload(iota_col_d.ap(), [P, 1], "iota_col")

            # resident node block + batch ids
            xres = cst.tile([P, NJ_N * D], F32, tag="xres")
            xres3 = xres[:].rearrange("p (j d) -> p j d", d=D)
            for c in range(7):  # 98 = 7 * 14
                sl = slice(c * 14, (c + 1) * 14)
                nc.sync.dma_start(xres3[:, sl, :], x3[:, sl, :])
            bidcols = cst.tile([P, NJ_N], F32, tag="bidcols")
            nc.sync.dma_start(bidcols[:], bid_d.ap().rearrange("(j p) -> p j", p=P))

            gate_v = cst.tile([S, D], F32, tag="gate_v")
            gate_e = cst.tile([S, D], F32, tag="gate_e")

            # ---- phase 1: partial segsum (transposed) + AllReduce + MLPs ----
            with (
                tc.tile_pool(name="p1", bufs=4) as p1,
                tc.tile_pool(name="p1ps", bufs=1, space="PSUM") as pp1,
                tc.tile_pool(name="mlpps", bufs=2, space="PSUM") as ppm,
            ):
                seg_ps = pp1.tile([P, S], F32, tag="seg_ps")
                for j in range(NJ_N):
                    oh = p1.tile([P, S], F32, tag="oh")
                    nc.vector.tensor_tensor(
                        oh[:],
                        bidcols[:, j : j + 1].to_broadcast([P, S]),
                        iota_row[:],
                        ALU.is_equal,
                    )
                    nc.tensor.matmul(
                        seg_ps[:],
                        xres3[:, j, :],
                        oh[:],
                        start=(j == 0),
                        stop=(j == NJ_N - 1),
                    )
                segT = p1.tile([P, S], F32, tag="segT")
                nc.vector.tensor_copy(segT[:], seg_ps[:])
                ccin = dpool.tile([P, S], F32, tag="ccin")
                ccout = dpool.tile([P, S], F32, tag="ccout")
                nc.gpsimd.dma_start(ccin[:], segT[:])
                nc.gpsimd.collective_compute(
                    "AllReduce",
                    ALU.add,
                    replica_groups=[list(range(NC))],
                    ins=[ccin.opt()],
                    outs=[ccout.opt()],
                )
                segS = p1.tile([P, S], F32, tag="segS")
                nc.gpsimd.dma_start(segS[:], ccout[:])
                cT = p1.tile([P, S], F32, tag="cT")
                nc.vector.tensor_tensor(cT[:], segS[:], inv_rep[:], ALU.mult)

                def mlp(h, g):
                    for i in (1, 2, 3):
                        ps = ppm.tile([P, S], F32, tag="mlp_ps")
                        nc.tensor.matmul(ps[:], ws[f"{g}w{i}"][:], h[:], start=True, stop=True)
                        hn = p1.tile([P, S], F32, tag=f"h_{g}{i}")
                        nc.scalar.activation(
                            hn[:], ps[:],
                            AF.Sigmoid if i == 3 else AF.Relu,
                            bias=bs[f"{g}b{i}"][:],
                        )
                        h = hn
                    return h

                gate_vT = mlp(cT, "v")
                c2T = p1.tile([P, S], F32, tag="c2T")
                nc.vector.tensor_tensor(c2T[:], cT[:], gate_vT[:], ALU.mult)
                gate_eT = mlp(c2T, "e")

                gate_terms = {}
                for gT, g, nm in ((gate_vT, gate_v, "v"), (gate_eT, gate_e, "e")):
                    pt = ppm.tile([S, P], F32, tag="tp_ps")
                    nc.tensor.transpose(pt[:], gT[:], ident[:])
                    nc.vector.tensor_copy(g[:], pt[:])
                    # split fp32 gate into 3 bf16 terms (exact to ~2^-24) so
                    # the gather matmuls run at bf16 rate instead of fp32
                    # LOW_HIGH emulation
                    terms = []
                    resid = g
                    for t in range(3):
                        gt = cst.tile([S, P], BF16, tag=f"g{nm}{t}")
                        nc.vector.tensor_copy(gt[:], resid[:])
                        if t < 2:
                            r = p1.tile([S, P], F32, tag=f"r{nm}{t}")
                            nc.vector.tensor_tensor(r[:], resid[:], gt[:], ALU.subtract)
                            resid = r
                        terms.append(gt)
                    gate_terms[nm] = terms

            # ---- phase 2/3: gating (nodes reuse resident x, edges stream) ----
            with (
                tc.tile_pool(name="io", bufs=6) as iop,
                tc.tile_pool(name="out", bufs=3) as outp,
                tc.tile_pool(name="mid", bufs=3) as midp,
                tc.tile_pool(name="pg", bufs=2, space="PSUM") as pgp,
            ):
                def gating(megas, terms, ids_dram, src3, out3, resident):
                    j0 = 0
                    for J in megas:
                        C = J * D
                        if resident is None:
                            xt = iop.tile([P, MEGA * D], F32, tag="in")
                            xt3 = xt[:].rearrange("p (j d) -> p j d", d=D)
                            nc.sync.dma_start(xt3[:, 0:J, :], src3[:, j0 : j0 + J, :])
                            src = xt[:, 0:C]
                        else:
                            src = resident[:, j0 * D : j0 * D + C]
                        # broadcast the bf16 seg-ids row to 32 partitions via DMA
                        bc = midp.tile([S, MEGA * D], BF16, tag="bc")
                        nc.sync.dma_start(
                            bc[:, 0:C],
                            ids_dram.ap()[None, j0 * D : j0 * D + C].to_broadcast([S, C]),
                        )
                        # one-hot: oht[s, e] = (ids[e] == s), bf16 (exact 0/1)
                        oht = midp.tile([S, MEGA * D], BF16, tag="oht")
                        nc.vector.tensor_scalar(
                            oht[:, 0:C], bc[:, 0:C], iota_col[0:S, :], None, ALU.is_equal
                        )
                        outt = outp.tile([P, MEGA * D], F32, tag="out")
                        pg = pgp.tile([P, MEGA * D], F32, tag="pg")
                        for jj in range(J):
                            jsl = slice(jj * D, (jj + 1) * D)
                            for t in range(3):
                                nc.tensor.matmul(
                                    pg[:, jsl],
                                    oht[:, jsl],
                                    terms[t][:],
                                    start=(t == 0),
                                    stop=(t == 2),
                                )
                        nc.vector.tensor_tensor(
                            outt[:, 0:C], src[:, 0:C], pg[:, 0:C], ALU.mult
                        )
                        ot3 = outt[:].rearrange("p (j d) -> p j d", d=D)
                        nc.scalar.dma_start(out3[:, j0 : j0 + J, :], ot3[:, 0:J, :])
                        j0 += J

                gating(_megas(NJ_N), gate_terms["v"], bidb_d, None, ox3, xres)
                gating(_megas(NJ_E), gate_terms["e"], segb_d, ea3, oe3, None)

    nc.compile()
    return nc


_NC_CACHE = None
LAST_RESULT = None


def _get_nc():
    global _NC_CACHE
    if _NC_CACHE is None:
        _NC_CACHE = _build()
    return _NC_CACHE


def _shard_starts(total, block):
    starts, cover = [], 0
    for k in range(NC):
        s = min(k * block, total - block)
        starts.append(s)
        cover = max(cover, s + block)
    assert cover == total
    return starts


def kernel(dst_na, ea, edge_idx, batch_id,
           Vg_W1, Vg_b1, Vg_W2, Vg_b2, Vg_W3, Vg_b3,
           Eg_W1, Eg_b1, Eg_W2, Eg_b2, Eg_W3, Eg_b3):
    dst_na = np.ascontiguousarray(np.asarray(dst_na, dtype=np.float32))
    ea = np.ascontiguousarray(np.asarray(ea, dtype=np.float32))
    bid = np.asarray(batch_id).astype(np.int64)
    eidx0 = np.asarray(edge_idx)[0].astype(np.int64)

    seg_e = bid[eidx0].astype(np.float32)
    bid_f = bid.astype(np.float32)

    cnt = np.bincount(bid, minlength=S).astype(np.float32)
    inv = 1.0 / np.maximum(cnt, 1.0)
    inv_rep = np.tile(inv[None, :], (P, 1)).astype(np.float32)

    consts = {
        "inv_rep": inv_rep,
        "ident": np.eye(P, dtype=np.float32),
        "iota_row": np.tile(np.arange(S, dtype=np.float32)[None, :], (P, 1)),
        "iota_col": np.arange(P, dtype=np.float32)[:, None].copy(),
        "vw1": np.ascontiguousarray(Vg_W1, np.float32),
        "vw2": np.ascontiguousarray(Vg_W2, np.float32),
        "vw3": np.ascontiguousarray(Vg_W3, np.float32),
        "ew1": np.ascontiguousarray(Eg_W1, np.float32),
        "ew2": np.ascontiguousarray(Eg_W2, np.float32),
        "ew3": np.ascontiguousarray(Eg_W3, np.float32),
        "vb1": np.asarray(Vg_b1, np.float32).reshape(D, 1).copy(),
        "vb2": np.asarray(Vg_b2, np.float32).reshape(D, 1).copy(),
        "vb3": np.asarray(Vg_b3, np.float32).reshape(D, 1).copy(),
        "eb1": np.asarray(Eg_b1, np.float32).reshape(D, 1).copy(),
        "eb2": np.asarray(Eg_b2, np.float32).reshape(D, 1).copy(),
        "eb3": np.asarray(Eg_b3, np.float32).reshape(D, 1).copy(),
    }

    nstarts = _shard_starts(N_NODES, BN)
    estarts = _shard_starts(N_EDGES, BE)

    in_maps = []
    cover = 0
    novs = []
    for k in range(NC):
        ns, es = nstarts[k], estarts[k]
        bidk = bid_f[ns : ns + BN].copy()
        ov = max(0, cover - ns)  # rows already covered by earlier cores
        novs.append(ov)
        if ov:
            bidk[:ov] = -1.0  # exclude from segment sum; output discarded
        cover = max(cover, ns + BN)
        in_maps.append(
            {
                "x": dst_na[ns : ns + BN],
                "bid": bidk,
                "bid_bf": bidk.astype(ml_dtypes.bfloat16),
                "ea": ea[es : es + BE],
                "seg_bf": seg_e[es : es + BE].astype(ml_dtypes.bfloat16),
                **consts,
            }
        )

    nc = _get_nc()
    res = run_bass_kernel_spmd(nc, in_maps, core_ids=list(range(NC)))
    global LAST_RESULT
    LAST_RESULT = res

    out_x = np.empty((N_NODES, D), dtype=np.float32)
    out_e = np.empty((N_EDGES, D), dtype=np.float32)
    for k in range(NC):
        ns, es = nstarts[k], estarts[k]
        ov = novs[k]
        out_x[ns + ov : ns + BN] = res.results[k]["out_x"][ov:]
        out_e[es : es + BE] = res.results[k]["out_e"]
    return (out_x, out_e)


# revision 18
# speedup vs baseline: 3.9617x; 1.1042x over previous
"""Trainium2 Bass kernel for the gnn_message_passing "Context" problem.

Reference computation (N=100000 nodes, E=600000 edges, D=128, B=32 graphs):
    c_V    = scatter_mean(dst_na, batch_id)            # [B, D]
    gate_V = sigmoid(MLP3_V(c_V))                      # [B, D]
    out_n  = dst_na * gate_V[batch_id]                 # [N, D]
    c_V2   = scatter_mean(out_n, batch_id)             # == c_V * gate_V (gate const per segment)
    gate_E = sigmoid(MLP3_E(c_V2))                     # [B, D]
    out_e  = ea * gate_E[batch_id[edge_idx[0]]]        # [E, D]

Strategy: block-shard nodes/edges across 8 cores (overlap trick for
non-divisible sizes).  Each core computes a partial transposed segment sum
[D, B] with one-hot matmuls on the PE, AllReduce (tiny, [128,32]) combines
them, then the two 3-layer MLPs run on-device in transposed layout.  Gating
gathers gate rows via one-hot matmuls (PE) and multiplies on the vector
engine; the one-hot construction is spread across PE (broadcast outer
product), ACT ((seg-s)^2 via per-partition bias) and GpSimd (< 1 compare).
"""

import ml_dtypes
import numpy as np

import concourse.bacc as bacc
import concourse.mybir as mybir
import concourse.tile as tile
from concourse.bass_utils import run_bass_kernel_spmd

F32 = mybir.dt.float32
BF16 = mybir.dt.bfloat16
AF = mybir.ActivationFunctionType
ALU = mybir.AluOpType

N_NODES = 100000
N_EDGES = 600000
D = 128
S = 32          # segments (graphs)
NC = 8          # cores
P = 128

# per-core block sizes (multiples of 128)
BN = 12544      # nodes per core  (98 tiles of 128)
BE = 75008      # edges per core  (586 tiles of 128)
NJ_N = BN // P  # 98
NJ_E = BE // P  # 586
MEGA = 16       # j-tiles per mega tile (2048 rows)


def _megas(nj):
    m = [MEGA] * (nj // MEGA)
    if nj % MEGA:
        m.append(nj % MEGA)
    return m


def _build():
    nc = bacc.Bacc(
        "TRN2",
        target_bir_lowering=False,
        debug=False,
        enable_asserts=False,
        num_devices=NC,
    )

    def inp(name, shape):
        return nc.dram_tensor(name, shape, F32, kind="ExternalInput")

    x_d = inp("x", [BN, D])
    bid_d = inp("bid", [BN])
    bidb_d = nc.dram_tensor("bid_bf", [BN], BF16, kind="ExternalInput")
    ea_d = inp("ea", [BE, D])
    segb_d = nc.dram_tensor("seg_bf", [BE], BF16, kind="ExternalInput")
    inv_d = inp("inv_col", [P, 1])
    w_d = {f"{g}w{i}": inp(f"{g}w{i}", [D, D]) for g in "ve" for i in (1, 2, 3)}
    b_d = {f"{g}b{i}": inp(f"{g}b{i}", [D, 1]) for g in "ve" for i in (1, 2, 3)}
    ident_d = inp("ident", [P, P])
    iota_row_d = inp("iota_row", [P, S])
    iota_col_d = inp("iota_col", [P, 1])

    ox_d = nc.dram_tensor("out_x", [BN, D], F32, kind="ExternalOutput")
    oe_d = nc.dram_tensor("out_e", [BE, D], F32, kind="ExternalOutput")

    x3 = x_d.ap().rearrange("(j p) d -> p j d", p=P)
    ea3 = ea_d.ap().rearrange("(j p) d -> p j d", p=P)
    ox3 = ox_d.ap().rearrange("(j p) d -> p j d", p=P)
    oe3 = oe_d.ap().rearrange("(j p) d -> p j d", p=P)

    with tile.TileContext(nc) as tc:
        with (
            tc.tile_pool(name="const", bufs=1) as cst,
            tc.tile_pool(name="dram", bufs=1, space="DRAM") as dpool,
        ):
            # ---- constants ----
            def cload(ap, shape, tag):
                t = cst.tile(shape, F32, tag=tag)
                nc.sync.dma_start(t[:], ap)
                return t

            inv_col = cload(inv_d.ap(), [P, 1], "inv_col")
            ws = {k: cload(v.ap(), [D, D], k) for k, v in w_d.items()}
            bs = {k: cload(v.ap(), [D, 1], k) for k, v in b_d.items()}
            ident = cload(ident_d.ap(), [P, P], "ident")
            iota_row = cload(iota_row_d.ap(), [P, S], "iota_row")
            iota_col = cload(iota_col_d.ap(), [P, 1], "iota_col")

            # warm up the CC stream early so the real AllReduce later is cheap
            wrm_i = dpool.tile([1, 16], F32, tag="wrm_i")
            wrm_o = dpool.tile([1, 16], F32, tag="wrm_o")
            nc.gpsimd.dma_start(wrm_i[:], ident_d.ap()[0:1, 0:16])
            nc.gpsimd.collective_compute(
                "AllReduce",
                ALU.add,
                replica_groups=[list(range(NC))],
                ins=[wrm_i.opt()],
                outs=[wrm_o.opt()],
            )

            # resident node block + batch ids
            bidcols = cst.tile([P, NJ_N], F32, tag="bidcols")
            nc.sync.dma_start(bidcols[:], bid_d.ap().rearrange("(j p) -> p j", p=P))
            xres = cst.tile([P, NJ_N * D], F32, tag="xres")
            xres3 = xres[:].rearrange("p (j d) -> p j d", d=D)
            for c in range(7):  # 98 = 7 * 14
                sl = slice(c * 14, (c + 1) * 14)
                eng = nc.sync if c % 2 == 0 else nc.scalar
                eng.dma_start(xres3[:, sl, :], x3[:, sl, :])

            gate_v = cst.tile([S, D], F32, tag="gate_v")
            gate_e = cst.tile([S, D], F32, tag="gate_e")

            # ---- phase 1: partial segsum (transposed) + AllReduce + MLPs ----
            with (
                tc.tile_pool(name="p1", bufs=4) as p1,
                tc.tile_pool(name="p1ps", bufs=1, space="PSUM") as pp1,
                tc.tile_pool(name="mlpps", bufs=2, space="PSUM") as ppm,
            ):
                # partial segment sum: psum[s, d] += onehot_j^T @ x_j
                # (onehot as the stationary operand: LDW cost scales with its
                #  32 columns instead of x's 128)
                seg_ps = pp1.tile([S, D], F32, tag="seg_ps")
                for j in range(NJ_N):
                    oh = p1.tile([P, S], F32, tag="oh")
                    nc.vector.tensor_tensor(
                        oh[:],
                        bidcols[:, j : j + 1].to_broadcast([P, S]),
                        iota_row[:],
                        ALU.is_equal,
                    )
                    nc.tensor.matmul(
                        seg_ps[:],
                        oh[:],
                        xres3[:, j, :],
                        start=(j == 0),
                        stop=(j == NJ_N - 1),
                    )
                segT = p1.tile([S, D], F32, tag="segT")
                nc.vector.tensor_copy(segT[:], seg_ps[:])
                ccin = dpool.tile([S, D], F32, tag="ccin")
                ccout = dpool.tile([S, D], F32, tag="ccout")
                nc.gpsimd.dma_start(ccin[:], segT[:])
                nc.gpsimd.collective_compute(
                    "AllReduce",
                    ALU.add,
                    replica_groups=[list(range(NC))],
                    ins=[ccin.opt()],
                    outs=[ccout.opt()],
                )
                segS = p1.tile([S, D], F32, tag="segS")
                nc.gpsimd.dma_start(segS[:], ccout[:])
                # c[s, d] = segsum * (1 / count_s), then transpose to [d, s]
                c_sd = p1.tile([S, D], F32, tag="c_sd")
                nc.vector.tensor_scalar_mul(c_sd[:], segS[:], inv_col[0:S, :])
                ct_ps = ppm.tile([P, S], F32, tag="ct_ps")
                nc.tensor.transpose(ct_ps[:], c_sd[:], ident[0:S, 0:S])
                cT = p1.tile([P, S], F32, tag="cT")
                nc.vector.tensor_copy(cT[:], ct_ps[:])

                def mlp(h, g):
                    for i in (1, 2, 3):
                        ps = ppm.tile([P, S], F32, tag="mlp_ps")
                        nc.tensor.matmul(ps[:], ws[f"{g}w{i}"][:], h[:], start=True, stop=True)
                        hn = p1.tile([P, S], F32, tag=f"h_{g}{i}")
                        nc.scalar.activation(
                            hn[:], ps[:],
                            AF.Sigmoid if i == 3 else AF.Relu,
                            bias=bs[f"{g}b{i}"][:],
                        )
                        h = hn
                    return h

                gate_vT = mlp(cT, "v")
                c2T = p1.tile([P, S], F32, tag="c2T")
                nc.vector.tensor_tensor(c2T[:], cT[:], gate_vT[:], ALU.mult)
                gate_eT = mlp(c2T, "e")

                gate_terms = {}
                for gT, g, nm in ((gate_vT, gate_v, "v"), (gate_eT, gate_e, "e")):
                    pt = ppm.tile([S, P], F32, tag="tp_ps")
                    nc.tensor.transpose(pt[:], gT[:], ident[:])
                    nc.vector.tensor_copy(g[:], pt[:])
                    # split fp32 gate into 3 bf16 terms (exact to ~2^-24) so
                    # the gather matmuls run at bf16 rate instead of fp32
                    # LOW_HIGH emulation
                    terms = []
                    resid = g
                    for t in range(3):
                        gt = cst.tile([S, P], BF16, tag=f"g{nm}{t}")
                        nc.vector.tensor_copy(gt[:], resid[:])
                        if t < 2:
                            r = p1.tile([S, P], F32, tag=f"r{nm}{t}")
                            nc.vector.tensor_tensor(r[:], resid[:], gt[:], ALU.subtract)
                            resid = r
                        terms.append(gt)
                    gate_terms[nm] = terms

            # ---- phase 2/3: gating (nodes reuse resident x, edges stream) ----
            with (
                tc.tile_pool(name="io", bufs=8) as iop,
                tc.tile_pool(name="out", bufs=3) as outp,
                tc.tile_pool(name="mid", bufs=3) as midp,
                tc.tile_pool(name="pg", bufs=2, space="PSUM") as pgp,
            ):
                def gating(megas, terms, ids_dram, src3, out3, resident):
                    j0 = 0
                    for m, J in enumerate(megas):
                        ld = nc.sync if m % 2 == 0 else nc.scalar
                        st_eng = nc.scalar if m % 2 == 0 else nc.sync
                        C = J * D
                        if resident is None:
                            xt = iop.tile([P, MEGA * D], F32, tag="in")
                            xt3 = xt[:].rearrange("p (j d) -> p j d", d=D)
                            ld.dma_start(xt3[:, 0:J, :], src3[:, j0 : j0 + J, :])
                            src = xt[:, 0:C]
                        else:
                            src = resident[:, j0 * D : j0 * D + C]
                        # broadcast the bf16 seg-ids row to 32 partitions via DMA
                        bc = midp.tile([S, MEGA * D], BF16, tag="bc")
                        st_eng.dma_start(
                            bc[:, 0:C],
                            ids_dram.ap()[None, j0 * D : j0 * D + C].to_broadcast([S, C]),
                        )
                        # one-hot: oht[s, e] = (ids[e] == s), bf16 (exact 0/1)
                        oht = midp.tile([S, MEGA * D], BF16, tag="oht")
                        nc.vector.tensor_scalar(
                            oht[:, 0:C], bc[:, 0:C], iota_col[0:S, :], None, ALU.is_equal
                        )
                        outt = outp.tile([P, MEGA * D], F32, tag="out")
                        pg = pgp.tile([P, MEGA * D], F32, tag="pg")
                        for jj in range(J):
                            jsl = slice(jj * D, (jj + 1) * D)
                            for t in range(3):
                                nc.tensor.matmul(
                                    pg[:, jsl],
                                    oht[:, jsl],
                                    terms[t][:],
                                    start=(t == 0),
                                    stop=(t == 2),
                                )
                        nc.vector.tensor_tensor(
                            outt[:, 0:C], src[:, 0:C], pg[:, 0:C], ALU.mult
                        )
                        ot3 = outt[:].rearrange("p (j d) -> p j d", d=D)
                        st_eng.dma_start(out3[:, j0 : j0 + J, :], ot3[:, 0:J, :])
                        j0 += J

                gating(_megas(NJ_N), gate_terms["v"], bidb_d, None, ox3, xres)
                gating(_megas(NJ_E), gate_terms["e"], segb_d, ea3, oe3, None)

    nc.compile()
    return nc


_NC_CACHE = None
LAST_RESULT = None


def _get_nc():
    global _NC_CACHE
    if _NC_CACHE is None:
        _NC_CACHE = _build()
    return _NC_CACHE


def _shard_starts(total, block):
    starts, cover = [], 0
    for k in range(NC):
        s = min(k * block, total - block)
        starts.append(s)
        cover = max(cover, s + block)
    assert cover == total
    return starts


def kernel(dst_na, ea, edge_idx, batch_id,
           Vg_W1, Vg_b1, Vg_W2, Vg_b2, Vg_W3, Vg_b3,
           Eg_W1, Eg_b1, Eg_W2, Eg_b2, Eg_W3, Eg_b3):
    dst_na = np.ascontiguousarray(np.asarray(dst_na, dtype=np.float32))
    ea = np.ascontiguousarray(np.asarray(ea, dtype=np.float32))
    bid = np.asarray(batch_id).astype(np.int64)
    eidx0 = np.asarray(edge_idx)[0].astype(np.int64)

    seg_e = bid[eidx0].astype(np.float32)
    bid_f = bid.astype(np.float32)

    cnt = np.bincount(bid, minlength=S).astype(np.float32)
    inv = 1.0 / np.maximum(cnt, 1.0)

    consts = {
        "inv_col": np.pad(inv, (0, P - S)).astype(np.float32)[:, None].copy(),
        "ident": np.eye(P, dtype=np.float32),
        "iota_row": np.tile(np.arange(S, dtype=np.float32)[None, :], (P, 1)),
        "iota_col": np.arange(P, dtype=np.float32)[:, None].copy(),
        "vw1": np.ascontiguousarray(Vg_W1, np.float32),
        "vw2": np.ascontiguousarray(Vg_W2, np.float32),
        "vw3": np.ascontiguousarray(Vg_W3, np.float32),
        "ew1": np.ascontiguousarray(Eg_W1, np.float32),
        "ew2": np.ascontiguousarray(Eg_W2, np.float32),
        "ew3": np.ascontiguousarray(Eg_W3, np.float32),
        "vb1": np.asarray(Vg_b1, np.float32).reshape(D, 1).copy(),
        "vb2": np.asarray(Vg_b2, np.float32).reshape(D, 1).copy(),
        "vb3": np.asarray(Vg_b3, np.float32).reshape(D, 1).copy(),
        "eb1": np.asarray(Eg_b1, np.float32).reshape(D, 1).copy(),
        "eb2": np.asarray(Eg_b2, np.float32).reshape(D, 1).copy(),
        "eb3": np.asarray(Eg_b3, np.float32).reshape(D, 1).copy(),
    }

    nstarts = _shard_starts(N_NODES, BN)
    estarts = _shard_starts(N_EDGES, BE)

    in_maps = []
    cover = 0
    novs = []
    for k in range(NC):
        ns, es = nstarts[k], estarts[k]
        bidk = bid_f[ns : ns + BN].copy()
        ov = max(0, cover - ns)  # rows already covered by earlier cores
        novs.append(ov)
        if ov:
            bidk[:ov] = -1.0  # exclude from segment sum; output discarded
        cover = max(cover, ns + BN)
        in_maps.append(
            {
                "x": dst_na[ns : ns + BN],
                "bid": bidk,
                "bid_bf": bidk.astype(ml_dtypes.bfloat16),
                "ea": ea[es : es + BE],
                "seg_bf": seg_e[es : es + BE].astype(ml_dtypes.bfloat16),
                **consts,
            }
        )

    nc = _get_nc()
    res = run_bass_kernel_spmd(nc, in_maps, core_ids=list(range(NC)))
    global LAST_RESULT
    LAST_RESULT = res

    out_x = np.empty((N_NODES, D), dtype=np.float32)
    out_e = np.empty((N_EDGES, D), dtype=np.float32)
    for k in range(NC):
        ns, es = nstarts[k], estarts[k]
        ov = novs[k]
        out_x[ns + ov : ns + BN] = res.results[k]["out_x"][ov:]
        out_e[es : es + BE] = res.results[k]["out_e"]
    return (out_x, out_e)
